# revision 38
# baseline (speedup 1.0000x reference)
"""Single-head attention on 8 Trainium2 NeuronCores (v3).

Problem: x[8, 2048, 768], Wq/Wk/Wv[768, 64]+biases, mask[2048, 2048] int32
Output:  softmax(mask(Q K^T / 8)) V   -> [8, 2048, 64] f32

Sharding: data-parallel over batch - core b computes batch element b.

Per-core dataflow (all matmuls bf16 in / fp32 psum accumulate):
  proj:  QK[128, n] = Wqk.T @ xT per 512-col group; bias added as a k=1
         matmul (lhsT=bias row, rhs=ones row). Q copied to both partition
         halves of qq; K scattered into kk by chunk parity (even chunks at
         partitions 0:64, odd at 64:128) so K needs no duplication for the
         row-tiled score matmuls. V as [keys,64] + ones column -> vp, so the
         PV matmul also yields the softmax denominator for free.
  loop:  per (sweep s of 1024 q-cols, key chunk k): one row-tiled score
         matmul pair -> st psum, one [128,1024] exp on ACT (the critical
         engine: ~33us total at 1 col/cycle), mask multiply on DVE (bf16 2x
         mode), two PV matmuls accumulating OT[65, 1024]; PV for chunk k-1
         is emitted alongside scores for chunk k so the PE stream never
         stalls on the exp/mult latency. Remaining projections are wedged
         into sweep 0's early chunks.
  out:   OT psum -> sbuf -> DMA; host does out[q,h] = OT[h,q]/OT[64,q].

DMAs all ride the SP hardware-DGE queue in consumption order; the mask is
moved in 8 big strided DMAs (4 chunks x 1024 cols each) to keep the issue
cost low (~565ns per DMA on the SP sequencer).
"""

import os

import numpy as np
import ml_dtypes

import bass_rust
import concourse.bass as bass
import concourse.mybir as mybir
import concourse.tile as tile
from concourse.bass_utils import run_bass_kernel_spmd

BF16 = ml_dtypes.bfloat16
F32 = mybir.dt.float32
BF = mybir.dt.bfloat16

N_CORES = 8
SEQ = 2048
WIDTH = 768
HEAD = 64
NCH = WIDTH // 128      # 6 contraction chunks for the projections
NKC = SEQ // 128        # 16 key chunks
QT = 1024               # q tile (columns processed per main-loop sweep)
NQT = SEQ // QT


def _split_excess_waits(nc, max_waits=1):
    """walrus in this container rejects >1 sync wait per instruction; hoist
    extras onto preceding same-engine NoOps (same semantics: the engine
    executes its stream in order, so waiting earlier is equivalent)."""
    n = 0
    for bb in nc.main_func.blocks:
        new_list = []
        for ins in bb.instructions:
            si = ins.sync_info
            if si is not None and len(si.on_wait) > max_waits:
                waits = list(si.on_wait)
                extra, keep = waits[:-max_waits], waits[-max_waits:]
                for j, w in enumerate(extra):
                    nop = bass_rust.InstNoOp(
                        name=f"{ins.name}-ws{j}", engine=ins.engine, ins=[], outs=[]
                    )
                    nop.sync_info = mybir.SyncInfo(on_wait=[w], on_update=[])
                    new_list.append(nop)
                    n += 1
                ins.sync_info = mybir.SyncInfo(
                    on_wait=keep, on_update=list(si.on_update)
                )
            new_list.append(ins)
        bb.instructions = new_list
    return n


def _strip_tail(nc):
    """Drop the NRT pseudo-sync ISA op and the second all-engine barrier that
    TileContext emits after the semaphore reset - ~4-5us of fixed tail. The
    final DMA-drain + first barrier + sem reset are kept, so re-execution of
    the NEFF still starts from clean semaphores."""
    for bb in nc.main_func.blocks:
        ins_list = list(bb.instructions)
        idx = None
        for i, ins in enumerate(ins_list):
            if getattr(ins, "is_reset_sema", False):
                idx = i
        if idx is not None and idx > len(ins_list) - 20:
            bb.instructions = ins_list[:idx + 1]
    return nc




def _hoist_dmas(nc, max_hoist=3):
    """Move wait-free SP-queue input-DMA issues to the front of the program.
    The TileContext/Bass entry sequence (two all-engine barriers, register
    restores) costs ~7.5us before the first DMA would otherwise issue; the
    input DMAs have no dependencies, and per-engine instruction order is all
    that codegen cares about, so issuing them first lets the transfers run
    during the preamble. DMA-completion semaphores only ever increase and
    consumers wait on >= thresholds, so early completion is harmless."""
    blocks = nc.main_func.blocks
    body = None
    for bb in blocks:
        if any(type(ins).__name__ == "InstDMACopy" for ins in bb.instructions):
            body = bb
            break
    if body is None:
        return 0
    total = 0
    for eng, cap in ((mybir.EngineType.SP, max_hoist),
                     (mybir.EngineType.Activation, 3)):
        # leading prefix of wait-free DMAs per engine queue. Only a prefix
        # may move: DMAs sharing a ring semaphore must complete in queue
        # order, so nothing may be hoisted past a waiting DMA.
        front, rest = [], []
        stopped = False
        for ins in body.instructions:
            si = ins.sync_info
            if (not stopped and type(ins).__name__ == "InstDMACopy"
                    and ins.engine == eng and len(front) < cap):
                if si is None or len(si.on_wait) == 0:
                    front.append(ins)
                    continue
                stopped = True
            rest.append(ins)
        if not front:
            continue
        body.instructions = rest
        # insert into the ENTRY block before this engine's first instruction
        # so the transfers run during the runtime preamble.
        entry = blocks[0]
        pos = len(entry.instructions)
        for i, ins in enumerate(entry.instructions):
            if getattr(ins, "engine", None) == eng:
                pos = i
                break
        entry.instructions = (
            entry.instructions[:pos] + front + entry.instructions[pos:]
        )
        total += len(front)
    return total


def _hoist_warmups(nc):
    """Move the warmup matmuls into the entry block after the PE's barrier
    arrival (its $S[151]-increment DRAIN) so they run during the entry
    rendezvous. They wait only on the bqk DMA semaphore."""
    blocks = nc.main_func.blocks
    body = None
    for bb in blocks:
        if any(type(ins).__name__ == "InstMatmult" for ins in bb.instructions):
            body = bb
            break
    if body is None:
        return 0
    moved = [ins for ins in body.instructions
             if type(ins).__name__ == "InstMatmult"
             and getattr(ins, "outs", None) is not None
             and "warm" in str(ins.name)]
    # safer: identify by emission order - the first 10 matmuls in the body
    mms = [ins for ins in body.instructions if type(ins).__name__ == "InstMatmult"]
    moved = mms[:20]
    idset = set(id(m) for m in moved)
    body.instructions = [i for i in body.instructions if id(i) not in idset]
    entry = blocks[0]
    pos = None
    for i, ins in enumerate(entry.instructions):
        if (ins.engine == mybir.EngineType.PE
                and type(ins).__name__ == "InstDrain"):
            pos = i + 1
            break
    if pos is None:
        pos = len(entry.instructions) - 1
    entry.instructions = (
        entry.instructions[:pos] + moved + entry.instructions[pos:]
    )
    return len(moved)


def _build():
    nc = bass.Bass("TRN2", target_bir_lowering=False, debug=False,
                   num_devices=N_CORES)

    # partition-major host layouts: row p holds everything partition p needs,
    # so each DMA is 128 large contiguous descriptors.
    xT_d = nc.declare_dram_parameter("xT", [128, 4 * NCH * 512], BF, False).ap()
    wqk_d = nc.declare_dram_parameter("Wqk", [128, NCH * 128], BF, False).ap()
    wv_d = nc.declare_dram_parameter("Wv", [128, NCH * HEAD], BF, False).ap()
    bqk_d = nc.declare_dram_parameter("bqk", [1, 128], BF, False).ap()
    id_d = nc.declare_dram_parameter("ident", [64, 64], BF, False).ap()
    # mT layout: [p][(s*NKC + c)*1024 + j] = mask[s*1024+j, c*128+p]
    mT_d = nc.declare_dram_parameter("mT", [128, NKC * SEQ], BF, False).ap()
    ot_d = nc.declare_dram_parameter("ot", [HEAD + 1, SEQ], BF, True).ap()

    EXP = mybir.ActivationFunctionType.Exp
    COPY = mybir.ActivationFunctionType.Copy
    ESCALE = 0.125   # 1/sqrt(HEAD)

    with tile.TileContext(nc) as tc:
        with (
            tc.tile_pool(name="const", bufs=1) as const,
            tc.tile_pool(name="pp", bufs=6) as ppool,
            tc.tile_pool(name="ep", bufs=2) as epool,
            tc.tile_pool(name="stp", bufs=3, space="PSUM") as stp,
            tc.tile_pool(name="otp", bufs=1, space="PSUM") as otp,
        ):
            # ---- input DMAs, all SP queue, in consumption order ----
            wqk = const.tile([128, NCH, 128], BF)
            bqk = const.tile([1, 128], BF)
            wv = const.tile([128, NCH, HEAD], BF)
            xt = const.tile([128, 4, NCH, 512], BF)
            mt = const.tile([128, NQT, NKC, QT], BF)

            def xt_dma(t):
                nc.sync.dma_start(
                    out=xt[:, t, :, :],
                    in_=xT_d[:, t * NCH * 512:(t + 1) * NCH * 512],
                )

            def mask_dma(s, c0, nck, eng=None):
                # chunks c0..c0+nck of sweep s: contiguous in DRAM AND in mt,
                # so each of the 128 descriptor rows moves nck*2KB at once.
                src0 = (s * NKC + c0) * QT
                (eng or nc.sync).dma_start(
                    out=mt[:, s, c0:c0 + nck, :],
                    in_=mT_d[:, src0:src0 + nck * QT],
                )

            ident = const.tile([64, 64], BF)
            # SP queue: bqk (1-descriptor DGE warmup), wqk, xt1 issue during
            # the runtime preamble (hoisted); ACT queue: xt0 + first mask
            # block. Anything later issues after the entry barrier (~8us).
            nc.sync.dma_start(out=bqk, in_=bqk_d)
            nc.scalar.dma_start(out=ident, in_=id_d)
            nc.sync.dma_start(out=wqk, in_=wqk_d)
            xt_dma(1)
            nc.scalar.dma_start(        # xt0 in halves: proj_qk_a(0)
                out=xt[:, 0, 0:3, :],   # starts on chunks c0-c2 while
                in_=xT_d[:, 0:3 * 512]) # c3-c5 are still in flight
            nc.scalar.dma_start(
                out=xt[:, 0, 3:NCH, :],
                in_=xT_d[:, 3 * 512:NCH * 512])
            mask_dma(0, 0, 4, eng=nc.scalar)
            nc.sync.dma_start(out=wv, in_=wv_d)
            nc.sync.dma_start(      # xt groups 2+3 as one 12KB-row DMA
                out=xt[:, 2:4, :, :],
                in_=xT_d[:, 2 * NCH * 512:4 * NCH * 512],
            )
            mask_dma(0, 4, 4)
            mask_dma(0, 8, 4)
            mask_dma(0, 12, 4)
            for c0 in range(0, NKC, 4):
                mask_dma(1, c0, 4)

            # PE p-state warmup: ~3us of throwaway matmuls on bqk (the
            # first DMA to land). _hoist_warmups moves these into the entry
            # block between the PE's barrier-arrival and barrier-wait, so
            # the PE reaches full clock during the rendezvous without
            # delaying any other engine.
            warm = stp.tile([128, 128], F32, tag="st", name="warm")
            for _ in range(20):
                nc.tensor.matmul(warm, bqk, bqk, start=True, stop=True)
            ones = const.tile([1, 512], BF)
            nc.vector.memset(ones, 1.0)

            # ---- projection targets ----
            # qktmp[0:64] = Q^T, [64:128] = K^T, straight from psum. The
            # row-tiled score matmuls read it DIRECTLY: even chunks run on
            # PE rows 64:128 (K already there), odd chunks on rows 0:64
            # (Q already there). Only the complements need copies: Q
            # duplicated to rows 64:128 (qhi) and odd chunks' K to rows
            # 0:64 (kkodd).
            qhi = const.tile([128, SEQ], BF)            # [64:128] = Q^T copy
            kkodd = const.tile([64, NKC // 2, 128], BF)
            vp = const.tile([128, NKC, HEAD + 1], BF)   # V | ones column
            qktmp = const.tile([128, SEQ], BF)

            qk_ps_t = {}

            def proj_qk_a(t):
                qk_ps_t[t] = stp.tile([128, 512], F32, tag="st",
                                      name=f"qk_ps{t}")
                for c in range(3):
                    nc.tensor.matmul(qk_ps_t[t], wqk[:, c, :], xt[:, t, c, :],
                                     start=(c == 0), stop=False)

            def proj_qk_b(t):
                cols = slice(t * 512, (t + 1) * 512)
                qk_ps = qk_ps_t[t]
                for c in range(3, NCH):
                    nc.tensor.matmul(qk_ps, wqk[:, c, :], xt[:, t, c, :],
                                     start=False, stop=False)
                nc.tensor.matmul(   # +bias: bqk[m] * ones[n]
                    qk_ps, bqk[0:1, :], ones[0:1, 0:512],
                    start=False, stop=True,
                )
                nc.vector.tensor_copy(out=qktmp[:, cols], in_=qk_ps)
                # groups 0/1 feed the first score matmuls: their complements
                # stay on DVE (idle and ~2.6x faster than Pool); later groups
                # go to Pool so they never delay the mask multiplies
                ceng = nc.vector if t < 2 else nc.gpsimd
                ceng.tensor_copy(
                    out=qhi[64:128, cols], in_=qktmp[0:64, cols])
                for j in range(4):
                    kc = 4 * t + j          # absolute key chunk
                    if kc % 2 == 1:
                        ceng.tensor_copy(
                            out=kkodd[:, kc // 2, :],
                            in_=qktmp[64:128,
                                      t * 512 + j * 128:t * 512 + (j + 1) * 128],
                        )

            def proj_qk(t):
                proj_qk_a(t)
                proj_qk_b(t)

            vtmp = const.tile([64, SEQ], BF)    # V^T staging

            vt_ps_t = {}

            def proj_vt_a(t):
                vt_ps_t[t] = stp.tile([64, 512], F32, tag="st",
                                      name=f"vt_ps{t}")
                for c in range(3):
                    nc.tensor.matmul(vt_ps_t[t], wv[:, c, :], xt[:, t, c, :],
                                     start=(c == 0), stop=False)

            def proj_vt_b(t):
                cols = slice(t * 512, (t + 1) * 512)
                for c in range(3, NCH):
                    nc.tensor.matmul(vt_ps_t[t], wv[:, c, :], xt[:, t, c, :],
                                     start=False, stop=(c == NCH - 1))
                nc.vector.tensor_copy(out=vtmp[:, cols], in_=vt_ps_t[t])

            def proj_vtr(t):
                # transpose V^T -> V[keys, h] chunks on the PE (bf16 psum)
                tp_ps = stp.tile([128, 4, HEAD], BF, tag="st", name=f"tp_ps{t}")
                for j in range(4):
                    nc.tensor.transpose(
                        tp_ps[:, j, :],
                        vtmp[:, t * 512 + j * 128:t * 512 + (j + 1) * 128],
                        ident)
                lo = 4 * t
                nc.vector.tensor_copy(out=vp[:, lo:lo + 4, 0:HEAD], in_=tp_ps)

            def kk_ap(k):
                if k % 2 == 0:      # K^T lives in qktmp rows 64:128
                    return qktmp[64:128, k * 128:(k + 1) * 128]
                return kkodd[:, k // 2, :]

            def qq_half(k, gq):
                if k % 2 == 0:      # even chunks stream Q from the hi copy
                    return qhi[64:128, gq]
                return qktmp[0:64, gq]

            nc.vector.memset(vp[:, :, HEAD:HEAD + 1], 1.0)
            warm2 = stp.tile([128, 128], F32, tag="st", name="warm2")
            for _ in range(12):
                nc.tensor.matmul(warm2, bqk, bqk, start=True, stop=True)
            proj_qk(0)
            proj_qk(1)

            # ---- main loop ----
            for s in range(NQT):
                ot_ps = otp.tile([HEAD + 1, QT], F32, tag="ot", name=f"ot_ps{s}")
                prev = []
                for k in range(NKC + 1):
                    cur = []
                    if k < NKC:
                        # the last chunk runs in 512-wide halves so the
                        # mult/PV/copy tail chain starts half an exp earlier
                        split = (s == 1 and k == NKC - 1) or (s == 0 and k == 0)
                        st = stp.tile([128, QT], F32, tag="st", name=f"st{s}_{k}")
                        p = ppool.tile([128, QT], BF, tag="p", name=f"p{s}_{k}")
                        halves = ((0, 512), (512, 1024)) if split else ((0, QT),)
                        for lo, hi in halves:
                            for h in range(lo // 512, hi // 512):
                                gq = slice(s * QT + h * 512, s * QT + (h + 1) * 512)
                                nc.tensor.matmul(
                                    st[:, h * 512:(h + 1) * 512],
                                    kk_ap(k), qq_half(k, gq),
                                    start=True, stop=True,
                                )
                            nc.scalar.activation(
                                p[:, lo:hi], st[:, lo:hi], EXP, scale=ESCALE)
                            nc.vector.tensor_mul(
                                p[:, lo:hi], p[:, lo:hi], mt[:, s, k, lo:hi])
                            for h in range(lo // 512, hi // 512):
                                cur.append((k, p, h))
                        if s == 0:
                            # remaining projection work wedged into the PE
                            # stream while ACT digests the early chunks;
                            # sub-microsecond pieces so ACT never starves
                            # for more than one wedge
                            {0: lambda: (proj_vt_a(0), proj_vt_b(0)),
                             1: lambda: proj_vtr(0),
                             2: lambda: proj_vt_a(1),
                             3: lambda: proj_vt_b(1),
                             4: lambda: proj_vtr(1),
                             5: lambda: proj_qk_a(2),
                             6: lambda: proj_qk_b(2),
                             7: lambda: proj_vt_a(2),
                             8: lambda: proj_vt_b(2),
                             9: lambda: proj_vtr(2),
                             10: lambda: proj_qk_a(3),
                             11: lambda: (proj_qk_b(3), proj_vt_a(3)),
                             12: lambda: (proj_vt_b(3), proj_vtr(3)),
                             }.get(k, lambda: None)()
                    for (pk, p, h) in prev:
                        nc.tensor.matmul(
                            ot_ps[:, h * 512:(h + 1) * 512],
                            vp[:, pk, :], p[:, h * 512:(h + 1) * 512],
                            start=(pk == 0), stop=(pk == NKC - 1),
                        )
                        if s == 1 and pk == NKC - 1 and h == 0:
                            # drain the finished h0 half while h1's PV runs
                            ot_sb1 = epool.tile([HEAD + 1, QT], BF,
                                                tag="osb", name="ot_sb1")
                            nc.scalar.activation(
                                ot_sb1[:, 0:512], ot_ps[:, 0:512], COPY)
                            nc.sync.dma_start(
                                out=ot_d[:, QT:QT + 512], in_=ot_sb1[:, 0:512])
                    prev = cur

                # psum -> sbuf, then DMA out. Sweep 0's copy runs mid-loop
                # where ACT is saturated, so it goes on DVE; sweep 1's copy
                # is split ACT/DVE so both halves finish sooner.
                ot_sb = epool.tile([HEAD + 1, QT], BF, tag="osb", name=f"ot_sb{s}")
                if s == 0:
                    nc.vector.tensor_copy(out=ot_sb, in_=ot_ps)
                    nc.sync.dma_start(out=ot_d[:, 0:QT], in_=ot_sb)
                else:
                    # h0 was drained inside the final prev-loop (ot_sb1)
                    nc.vector.tensor_copy(
                        out=ot_sb[:, 512:1024], in_=ot_ps[:, 512:1024])
                    nc.sync.dma_start(
                        out=ot_d[:, QT + 512:SEQ], in_=ot_sb[:, 512:1024])

    if os.environ.get('ATTN_HOIST', '1') == '1':
        _hoist_dmas(nc)
        _hoist_warmups(nc)
    _split_excess_waits(nc)
    _strip_tail(nc)
    return nc


_CACHE = {}


def _get_nc():
    if "nc" not in _CACHE:
        _CACHE["nc"] = _build()
    return _CACHE["nc"]


def _prep_in_maps(x, Wq, bq, Wk, bk, Wv, bv, mask):
    x = np.asarray(x, dtype=np.float32)
    Wqk = np.concatenate(
        [np.asarray(Wq, np.float32), np.asarray(Wk, np.float32)], axis=1)
    # partition-major: row p holds [c0 cols | c1 cols | ...] for w = c*128+p
    Wqkh = np.ascontiguousarray(
        Wqk.reshape(NCH, 128, 128).transpose(1, 0, 2).reshape(128, NCH * 128)
    ).astype(BF16)
    Wvh = np.ascontiguousarray(
        np.asarray(Wv, np.float32).reshape(NCH, 128, HEAD)
        .transpose(1, 0, 2).reshape(128, NCH * HEAD)
    ).astype(BF16)
    bqk = np.concatenate(
        [np.asarray(bq, np.float32), np.asarray(bk, np.float32)]
    ).astype(BF16).reshape(1, 128)
    ident = np.eye(64, dtype=np.float32).astype(BF16)

    # mTh[p, (s*NKC+c)*1024 + j] = mask[s*1024+j, c*128+p]
    mTh = np.ascontiguousarray(
        np.asarray(mask, np.float32).T.reshape(NKC, 128, NQT, QT)
        .transpose(1, 2, 0, 3).reshape(128, NKC * SEQ)
    ).astype(BF16)
    in_maps = []
    for b in range(N_CORES):
        # xth[p, t, c, j] = x[b][t*512+j, c*128+p]
        xth = np.ascontiguousarray(
            x[b].reshape(4, 512, NCH, 128).transpose(3, 0, 2, 1)
            .reshape(128, 4 * NCH * 512)
        ).astype(BF16)
        in_maps.append({
            "xT": xth, "Wqk": Wqkh, "Wv": Wvh, "bqk": bqk, "ident": ident,
            "mT": mTh,
        })
    return in_maps


def _run(in_maps, trace=False, **kw):
    nc = _get_nc()
    return run_bass_kernel_spmd(nc, in_maps, list(range(N_CORES)), trace=trace, **kw)


def kernel(x, Wq, bq, Wk, bk, Wv, bv, mask):
    in_maps = _prep_in_maps(x, Wq, bq, Wk, bk, Wv, bv, mask)
    res = _run(in_maps)
    out = np.empty((N_CORES, SEQ, HEAD), np.float32)
    bvf = np.asarray(bv, np.float32)   # softmax weights sum to 1, so the V
    for b in range(N_CORES):           # bias is just an additive constant
        ot = np.asarray(res.results[b]["ot"]).astype(np.float32)
        out[b] = (ot[:HEAD] / ot[HEAD:HEAD + 1]).T + bvf
    return out
